# revision 3
# baseline (speedup 1.0000x reference)
"""AVRoPE (axial video RoPE + audio token) Trainium2 Bass kernel.

Problem (hardcoded shapes):
  x_video: (4, 16, 4096, 96) f32   [B, H, NF*P*P, D] with NF=P=16, D=96
  x_audio: (4, 16, 16, 96)   f32
  cos/sin: (16, 17, 17, 48)  f32
  offset:  scalar int (always 0 here since cos first dim == NF)

Reference semantics (per (b,h) pair, fully elementwise):
  video token t=(n,y,x), y,x<16:  x0=x[0::2], x1=x[1::2]
     y0 = x0*c(n,y,x) - x1*s(n,y,x);  y1 = x1*c + x0*s;  out = [y0, y1]
  audio token n: same with c(n,16,16), s(n,16,16).

Sharding: data-parallel over flattened (B*H)=64 -> 8 (b,h) pairs per core.
cos/sin replicated; no cross-core comms.

Device layout per core:
  xv  (32768, 96) viewed as 8 tiles of (128, 3072): partition j of tile i
      holds tokens [i*4096 + j*32, ... +32) (32 tokens * 96 ch, contiguous).
  Each video tile i covers exactly one (b,h)'s 4096 tokens, so one resident
  cos/sin tile pair (128, 3072) serves all 8 tiles.
  Host precomputes:
    cv: per-token halves layout [c_0..c_47, c_0..c_47]        (4096, 96)
    sv: per-token interleaved signed [+s_0, -s_0, +s_1, ...]  (4096, 96)
  Per tile (4 DVE tensor_tensor ops, strided APs do the deinterleave):
    y(k,h,c)   = x(k, 2c+h) * cv(k, 48h+c)        # [x0*c | x1*c]
    q          = x * sv                            # [+x0*s, -x1*s, ...]
    y(k,0,c)  += q(k, 2c+1)                        # y0 = x0*c - x1*s
    y(k,1,c)  += q(k, 2c+0)                        # y1 = x1*c + x0*s
  Audio: one (128, 96) tile per core (rows = bh*16 + n), same 4 ops.
"""

import numpy as np

B, H, NF, P, D = 4, 16, 16, 16, 96
BH = B * H
N_CORES = 8
BH_PER_CORE = BH // N_CORES          # 8
LV = NF * P * P                      # 4096 video tokens per (b,h)
HALF = D // 2                        # 48
K = 32                               # tokens per partition per video tile
FD = K * D                           # 3072 free-dim elements per partition
VTILES = BH_PER_CORE * LV // (128 * K)   # 8

_NC_CACHE = {}


def _build_nc(use_gpsimd_adds=False):
    import concourse.bass as bass
    import concourse.bacc as bacc
    import concourse.mybir as mybir
    from concourse.tile import TileContext
    from contextlib import ExitStack

    f32 = mybir.dt.float32
    mult = mybir.AluOpType.mult
    add = mybir.AluOpType.add

    nc = bacc.Bacc("TRN2", target_bir_lowering=False, debug=False)
    xv = nc.declare_dram_parameter("xv", [BH_PER_CORE * LV, D], f32, isOutput=False)
    xa = nc.declare_dram_parameter("xa", [128, D], f32, isOutput=False)
    cv = nc.declare_dram_parameter("cv", [LV, D], f32, isOutput=False)
    sv = nc.declare_dram_parameter("sv", [LV, D], f32, isOutput=False)
    ca = nc.declare_dram_parameter("ca", [128, D], f32, isOutput=False)
    sa = nc.declare_dram_parameter("sa", [128, D], f32, isOutput=False)
    yv = nc.declare_dram_parameter("yv", [BH_PER_CORE * LV, D], f32, isOutput=True)
    ya = nc.declare_dram_parameter("ya", [128, D], f32, isOutput=True)

    xv_t = xv.rearrange("(t p k) d -> t p (k d)", t=VTILES, p=128, k=K)
    yv_t = yv.rearrange("(t p k) d -> t p (k d)", t=VTILES, p=128, k=K)
    cv_t = cv.rearrange("(p k) d -> p (k d)", p=128, k=K)
    sv_t = sv.rearrange("(p k) d -> p (k d)", p=128, k=K)

    with TileContext(nc) as tc, ExitStack() as ctx:
        const = ctx.enter_context(tc.tile_pool(name="const", bufs=1))
        cvt = const.tile([128, FD], f32, tag="cvt")
        svt = const.tile([128, FD], f32, tag="svt")
        cat = const.tile([128, D], f32, tag="cat")
        sat = const.tile([128, D], f32, tag="sat")
        nc.sync.dma_start(cvt[:], cv_t)
        nc.sync.dma_start(svt[:], sv_t)
        nc.sync.dma_start(cat[:], ca[:, :])
        nc.sync.dma_start(sat[:], sa[:, :])

        xp = ctx.enter_context(tc.tile_pool(name="x", bufs=3))
        yp = ctx.enter_context(tc.tile_pool(name="y", bufs=3))
        qp = ctx.enter_context(tc.tile_pool(name="q", bufs=3))

        def rope_ops(xt, yt, qt, ct, st, k, add_engine):
            # x in interleaved order: idx(k, h, c) = k*96 + 2c + h
            x_de = xt[:].rearrange("p (k c h) -> p k h c", k=k, c=HALF, h=2)
            # y / c in halves order: idx(k, h, c) = k*96 + 48h + c
            y4 = yt[:].rearrange("p (k h c) -> p k h c", k=k, h=2, c=HALF)
            c4 = ct[:].rearrange("p (k h c) -> p k h c", k=k, h=2, c=HALF)
            q4 = qt[:].rearrange("p (k c h) -> p k h c", k=k, c=HALF, h=2)
            nc.vector.tensor_tensor(y4, x_de, c4, mult)
            nc.vector.tensor_tensor(qt[:], xt[:], st, mult)
            add_engine.tensor_tensor(y4[:, :, 0, :], y4[:, :, 0, :], q4[:, :, 1, :], add)
            add_engine.tensor_tensor(y4[:, :, 1, :], y4[:, :, 1, :], q4[:, :, 0, :], add)

        add_engine = nc.gpsimd if use_gpsimd_adds else nc.vector

        for i in range(VTILES):
            xt = xp.tile([128, FD], f32, tag="x")
            nc.sync.dma_start(xt[:], xv_t[i])
            yt = yp.tile([128, FD], f32, tag="y")
            qt = qp.tile([128, FD], f32, tag="q")
            rope_ops(xt, yt, qt, cvt[:], svt[:], K, add_engine)
            nc.sync.dma_start(yv_t[i], yt[:])

        # audio: one small tile
        xat = xp.tile([128, D], f32, tag="xa")
        nc.sync.dma_start(xat[:], xa[:, :])
        yat = yp.tile([128, D], f32, tag="ya")
        qat = qp.tile([128, D], f32, tag="qa")
        rope_ops(xat, yat, qat, cat[:], sat[:], 1, nc.vector)
        nc.sync.dma_start(ya[:, :], yat[:])

    nc.finalize()
    return nc


def _get_nc(**kw):
    key = tuple(sorted(kw.items()))
    if key not in _NC_CACHE:
        _NC_CACHE[key] = _build_nc(**kw)
    return _NC_CACHE[key]


def _prep_consts(cos, sin, offset):
    c = np.asarray(cos, np.float32)[offset:offset + NF]   # (16,17,17,48)
    s = np.asarray(sin, np.float32)[offset:offset + NF]
    cvid = np.ascontiguousarray(c[:, :P, :P, :]).reshape(LV, HALF)
    svid = np.ascontiguousarray(s[:, :P, :P, :]).reshape(LV, HALF)
    cv = np.concatenate([cvid, cvid], axis=1)             # halves layout
    sv = np.empty((LV, D), np.float32)
    sv[:, 0::2] = svid
    sv[:, 1::2] = -svid
    caud = c[:, P, P, :]                                  # (16,48)
    saud = s[:, P, P, :]
    ca1 = np.concatenate([caud, caud], axis=1)            # (16,96)
    sa1 = np.empty((NF, D), np.float32)
    sa1[:, 0::2] = saud
    sa1[:, 1::2] = -saud
    ca = np.tile(ca1, (BH_PER_CORE, 1))                   # (128,96)
    sa = np.tile(sa1, (BH_PER_CORE, 1))
    return cv, sv, ca, sa


def _make_in_maps(x_video, x_audio, cos, sin, offset):
    x_video = np.ascontiguousarray(np.asarray(x_video), dtype=np.float32)
    x_audio = np.ascontiguousarray(np.asarray(x_audio), dtype=np.float32)
    off = int(np.asarray(offset))
    cv, sv, ca, sa = _prep_consts(cos, sin, off)
    xvf = x_video.reshape(BH, LV, D)
    xaf = x_audio.reshape(BH, NF, D)
    in_maps = []
    for c0 in range(N_CORES):
        sl = slice(c0 * BH_PER_CORE, (c0 + 1) * BH_PER_CORE)
        in_maps.append({
            "xv": xvf[sl].reshape(BH_PER_CORE * LV, D),
            "xa": xaf[sl].reshape(BH_PER_CORE * NF, D),
            "cv": cv, "sv": sv, "ca": ca, "sa": sa,
        })
    return in_maps


def _gather(results):
    yv = np.stack([np.asarray(r["yv"]).reshape(BH_PER_CORE, LV, D)
                   for r in results]).reshape(B, H, LV, D)
    ya = np.stack([np.asarray(r["ya"]).reshape(BH_PER_CORE, NF, D)
                   for r in results]).reshape(B, H, NF, D)
    return yv, ya


def run(x_video, x_audio, cos, sin, offset, trace=False, **nc_kw):
    from concourse.bass_utils import run_bass_kernel_spmd
    nc = _get_nc(**nc_kw)
    in_maps = _make_in_maps(x_video, x_audio, cos, sin, offset)
    res = run_bass_kernel_spmd(nc, in_maps, list(range(N_CORES)), trace=trace)
    yv, ya = _gather(res.results)
    return (yv, ya), res


def kernel(x_video, x_audio, cos, sin, offset):
    (yv, ya), _ = run(x_video, x_audio, cos, sin, offset, trace=False)
    return yv, ya
